# revision 1
# baseline (speedup 1.0000x reference)
"""AdaptiveConv2DMod Trainium2 kernel.

Per-sample modulated 3x3 conv (StyleGAN2-style) on 8 NeuronCores,
data-parallel over batch (1 sample per core, no collectives).

Per-core layout:
  - Input rows stream through a circular SBUF "tape": image row r lives at
    partition group r%4 (32 channels each), slot (r//4)%NSLOT, width padded
    to W+2 with zero columns.
  - Conv = 3 x-taps (kx) x row-window matmuls: for each output row y, the
    contraction over (in-channel, ky) is a K<=96 matmul over partition
    groups holding rows y-1..y+1.  Rows 4t..4t+3 form 4 PSUM col groups of
    one PSUM bank -> one [128, W] evacuation copy per 4 rows.
  - Weights (per-sample softmax-mixed + modulated + demodulated) are built
    on-device, transposed to lhsT layout via PE transposes, and replicated
    into 4 row-alignment variants so any (row-window, tile_position) pair
    reads one contiguous AP.
  - Matmuls run in bf16 (inputs rounded via a DVE pass), PSUM/output f32.
"""

import os
import sys

import numpy as np

try:
    import concourse.bass as bass  # noqa: F401
except ImportError:
    sys.path.insert(0, "/opt/trn_rl_repo")

import concourse.bass as bass
import concourse.tile as tile
from concourse import bacc, mybir
from concourse.bass_utils import run_bass_kernel_spmd

F32 = mybir.dt.float32
F32R = mybir.dt.float32r
BF16 = mybir.dt.bfloat16

C = 32          # in/out channels
NK = 4          # kernel bank size
EPS = 1e-8


def build_graph(H=512, W=512, nslot=32, ablate="", timing=False, repeat=1):
    """Build the per-core Bass graph. Returns compiled Bacc.

    ablate: comma-set of stages to skip ("mm", "evac", "conv", "odma")
    for TimelineSim bottleneck analysis only.
    timing: write the image to an Internal DRAM scratch and expose only a
    tiny external output, so repeated executions can be queued back-to-back
    without device-memory pressure (wall-clock delta timing).
    """
    skip = set(ablate.split(",")) if ablate else set()
    T = H // 4                      # row groups
    nslot = min(nslot, T)
    Wp = W + 4                      # padded width (4B-aligned interior)

    nc = bacc.Bacc("TRN2", target_bir_lowering=False, debug=False)

    fmap = nc.dram_tensor("fmap", [C, H, W], F32, kind="ExternalInput")
    mod = nc.dram_tensor("mod", [1, C], F32, kind="ExternalInput")
    kmod = nc.dram_tensor("kernel_mod", [1, NK], F32, kind="ExternalInput")
    wbank = nc.dram_tensor("weights", [NK, C, C, 3, 3], F32, kind="ExternalInput")
    ident = nc.inline_tensor(np.eye(C, dtype=np.float32), name="ident32")
    if timing:
        out = nc.dram_tensor("out", [1, NK], F32, kind="ExternalOutput")
        oscr = nc.dram_tensor("oscr", [C, H, W], F32, kind="Internal")
        osink = oscr
    else:
        out = nc.dram_tensor("out", [C, H, W], F32, kind="ExternalOutput")
        osink = out

    # Block-strip DRAM views (per-DMA fixed cost ~2.4us -> batch 16 slots):
    # in:  per (block b, group g): rows 4*(16b+s)+g, s=0..15 -> [i, s, x]
    # out: per (block b, group c): rows 4*(16b+s)+c -> [o, s, x]
    SBLK = min(16, H // 4)          # slots per DMA block
    fm_bs = fmap.ap().rearrange("i (b s g) x -> b g i s x", s=SBLK, g=4)
    fm_half = fm_bs
    out_bs = osink.ap().rearrange("o (b s c) x -> b c o s x", s=SBLK, c=4)

    def pieces_for_row(y):
        """One (tape, g0, K, slot) piece covering rows y-1..y+1.

        Tape 0 (X): row r at group r%4 -- whole window in one slot for
        y%4 in {1,2}.  Tape 1 (X2): row r at group (r+2)%4 -- whole window
        in one slot for y%4 in {3,0}.  K=128 reads the unused 4th strip
        with zero weights; image-boundary rows shrink to K=64."""
        c = y % 4
        s = y // 4
        if c in (1, 2):
            return [(0, 0, 128, s % nslot)]
        if c == 3:
            if y == H - 1:
                return [(1, 0, 64, (s + 1) % nslot)]
            return [(1, 0, 128, (s + 1) % nslot)]
        if y == 0:
            return [(1, 2, 64, 0)]
        return [(1, 0, 128, s % nslot)]

    with tile.TileContext(nc) as tc:
        # ---------------- persistent pools ----------------
        with (
            tc.tile_pool(name="xpool", bufs=1) as xpool,
            tc.tile_pool(name="wpool", bufs=1) as wpool,
            tc.tile_pool(name="cpool", bufs=1) as cpool,
        ):
            X = xpool.tile([128, nslot * Wp], BF16)
            # phase-shifted tape: row r at group (r+2)%4, slot (r+2)//4.
            # windows for y%4 in {3,0} fit one slot here -> single K=128 MMs
            X2 = xpool.tile([128, nslot * Wp], BF16)
            Wt = wpool.tile([128, 384], BF16)      # 4 alignments x 3 kx x 32 o
            id32 = cpool.tile([C, C], F32)
            ones1 = cpool.tile([1, C], F32)

            nc.sync.dma_start(id32[:, :], ident.ap())
            nc.gpsimd.memset(ones1[:, :], 1.0)
            # unused (g, alignment) weight strips must be zero: K=128 matmuls
            # read all 4 strips and rely on zero weights for the extra row
            nc.vector.memset(Wt[:, :], 0.0)

            # zero the read pad columns of every slot, plus the final slot
            # (read once by block 0's X->X2 wrap copy before it is written)
            xv = X[:, :].rearrange("p (s q) -> p s q", q=Wp)
            nc.vector.memset(xv[:, :, 0:2], 0.0)
            nc.vector.memset(xv[:, :, Wp - 2:Wp], 0.0)
            nc.gpsimd.memset(X[:, (nslot - 1) * Wp:nslot * Wp], 0.0)

            # ---------------- weight preparation ----------------
            with (
                tc.tile_pool(name="prep", bufs=2) as prep,
                tc.tile_pool(name="prep_ps", bufs=2, space="PSUM") as prep_ps,
            ):
                # softmax(kernel_mod) -> attn [1, NK]
                km = prep.tile([1, NK], F32)
                nc.sync.dma_start(km[:, :], kmod.ap())
                mx = prep.tile([1, 1], F32)
                nc.vector.reduce_max(mx[:, :], km[:, :], axis=mybir.AxisListType.X)
                nmx = prep.tile([1, 1], F32)
                nc.scalar.mul(nmx[:, :], mx[:, :], -1.0)
                ex = prep.tile([1, NK], F32)
                nc.scalar.activation(
                    ex[:, :], km[:, :], mybir.ActivationFunctionType.Exp,
                    bias=nmx[:, 0:1],
                )
                sm = prep.tile([1, 1], F32)
                nc.vector.reduce_sum(sm[:, :], ex[:, :], axis=mybir.AxisListType.X)
                rs = prep.tile([1, 1], F32)
                nc.vector.reciprocal(rs[:, :], sm[:, :])
                attn = prep.tile([1, NK], F32)
                nc.vector.tensor_scalar_mul(attn[:, :], ex[:, :], rs[:, 0:1])

                # broadcast attn to all 32 partitions
                attnB = prep.tile([C, NK], F32)
                nc.gpsimd.partition_broadcast(attnB[:, :], attn[:, :])

                # P[o, n*288 + i*9 + tap] = weights[n, o, i, ky, kx]
                P = prep.tile([C, NK * 288], F32)
                nc.sync.dma_start(
                    P[:, :], wbank.ap().rearrange("n o i ky kx -> o n (i ky kx)")
                )

                # mix[o, i*9+tap] = sum_n attn[n] * P[o, n, ...]
                mix = prep.tile([C, 288], F32, tag="mix")
                tmp = prep.tile([C, 288], F32, tag="tmp")
                nc.vector.tensor_scalar_mul(mix[:, :], P[:, 0:288], attnB[:, 0:1])
                for n in range(1, NK):
                    nc.vector.tensor_scalar_mul(
                        tmp[:, :], P[:, n * 288:(n + 1) * 288], attnB[:, n:n + 1]
                    )
                    nc.vector.tensor_add(mix[:, :], mix[:, :], tmp[:, :])

                # mvec[i, 1] = mod + 1 ;  m2 = mvec^2
                mv = prep.tile([C, 1], F32, tag="mv")
                nc.sync.dma_start(mv[:, :], mod.ap().rearrange("a i -> i a"))
                m1 = prep.tile([C, 1], F32, tag="m1")
                nc.scalar.add(m1[:, :], mv[:, :], 1.0)
                m2 = prep.tile([C, 1], F32, tag="m2")
                nc.vector.tensor_mul(m2[:, :], m1[:, :], m1[:, :])

                # s[o, i] = sum_tap mix^2
                sq = prep.tile([C, 288], F32, tag="tmp")
                nc.vector.tensor_mul(sq[:, :], mix[:, :], mix[:, :])
                s_oi = prep.tile([C, C], F32, tag="soi")
                nc.vector.reduce_sum(
                    s_oi[:, :],
                    sq[:, :].rearrange("p (i t) -> p i t", t=9),
                    axis=mybir.AxisListType.X,
                )
                # sT[i, o]
                ps_a = prep_ps.tile([C, C], F32, tag="psa")
                nc.tensor.transpose(ps_a[:, :], s_oi[:, :], id32[:, :])
                sT = prep.tile([C, C], F32, tag="soi")
                nc.vector.tensor_copy(sT[:, :], ps_a[:, :])

                # normsq[1, o] = m2 . sT  (contract i)
                ps_n = prep_ps.tile([1, C], F32, tag="psa")
                nc.tensor.matmul(
                    ps_n[:, :], m2[:, :], sT[:, :], start=True, stop=True
                )
                ns = prep.tile([1, C], F32, tag="ns")
                nc.vector.tensor_scalar_max(ns[:, :], ps_n[:, :], EPS)
                sqn = prep.tile([1, C], F32, tag="sqn")
                nc.scalar.sqrt(sqn[:, :], ns[:, :])
                inv = prep.tile([1, C], F32, tag="inv")
                nc.vector.reciprocal(inv[:, :], sqn[:, :])

                # invT[o, 1] via PE transpose (identity [1,1] = ones)
                ps_i = prep_ps.tile([C, 1], F32, tag="psa")
                nc.tensor.transpose(ps_i[:, :], inv[:, :], ones1[:, 0:1])
                invT = prep.tile([C, 1], F32, tag="invT")
                nc.vector.tensor_copy(invT[:, :], ps_i[:, :])

                # wtA[o, i*9+tap] = mix * inv[o]
                wtA = prep.tile([C, 288], F32, tag="mix2")
                nc.vector.tensor_scalar_mul(wtA[:, :], mix[:, :], invT[:, 0:1])

                # 9 PE transposes -> psW[i, tap*32+o]
                ps_w = prep_ps.tile([C, 288], F32, tag="psw")
                wtA_t = wtA[:, :].rearrange("p (i t) -> p t i", t=9)
                for tap in range(9):
                    nc.tensor.transpose(
                        ps_w[:, tap * C:(tap + 1) * C], wtA_t[:, tap, :], id32[:, :]
                    )
                # wtB0[i, ky*96 + kx*32 + o] = psW * (1+mod[i])
                wtB0 = prep.tile([C, 288], BF16, tag="wtB0")
                nc.vector.tensor_scalar_mul(wtB0[:, :], ps_w[:, :], m1[:, 0:1])

                # replicate into 4 alignment variants:
                # Wt[32g+i, 96a + kxo] = wtB0[i, 96*((g-a)%4) + kxo]
                for a in range(4):
                    for ky in range(3):
                        g = (a + ky) % 4
                        nc.sync.dma_start(
                            Wt[32 * g:32 * g + 32, 96 * a:96 * a + 96],
                            wtB0[:, 96 * ky:96 * ky + 96],
                        )

            # ---------------- main conv loop ----------------
            with (
                tc.tile_pool(name="cps", bufs=8, space="PSUM") as cps,
                tc.tile_pool(name="opool", bufs=2) as opool,
                tc.tile_pool(name="spool", bufs=2) as spool,
            ):
                NBLK = T // SBLK

                def load_block(b):
                    stg = spool.tile([128, SBLK * W], F32, tag="stg")
                    for g in range(4):
                        nc.sync.dma_start(
                            stg[32 * g:32 * g + 32, :].rearrange(
                                "p (s x) -> p s x", x=W
                            ),
                            fm_bs[b, g],
                        )
                    return stg

                def convert_block(stg, b):
                    # f32 -> bf16 into X, 2 slots per DVE op
                    if "cvt" in skip:
                        return
                    for j in range(0, SBLK, 2):
                        p = (b * SBLK + j) % nslot
                        nc.vector.tensor_copy(
                            xv[:, p:p + 2, 2:2 + W],
                            stg[:, j * W:(j + 2) * W].rearrange(
                                "p (s x) -> p s x", x=W
                            ),
                        )

                def s2s_range(q0, nsl):
                    # X2 slots q0..q0+nsl from X (scalar HWDGE ring)
                    # X2 strip g <- X strip (g+2)%4; g in {0,1} shift -1 slot
                    for g in (2, 3):
                        nc.scalar.dma_start(
                            X2[32 * g:32 * g + 32,
                               q0 * Wp:(q0 + nsl) * Wp],
                            X[32 * (g - 2):32 * (g - 2) + 32,
                              q0 * Wp:(q0 + nsl) * Wp],
                        )
                    qm1 = (q0 - 1) % nslot
                    nc.scalar.dma_start(
                        X2[0:64, q0 * Wp:(q0 + 1) * Wp],
                        X[64:128, qm1 * Wp:(qm1 + 1) * Wp],
                    )
                    if nsl > 1:
                        nc.scalar.dma_start(
                            X2[0:64, (q0 + 1) * Wp:(q0 + nsl) * Wp],
                            X[64:128, q0 * Wp:(q0 + nsl - 1) * Wp],
                        )

                def s2s_block(b):
                    if "cvt" in skip:
                        return
                    s2s_range((SBLK * b) % nslot, SBLK)

                def s2s_epilogue():
                    # X2 slot 0 strips 0,1 <- last X slot strips 2,3
                    # (rows H-2, H-1 for the y=H-1 window)
                    if "cvt" in skip:
                        return
                    ql = (T - 1) % nslot
                    nc.scalar.dma_start(
                        X2[0:64, 0:Wp], X[64:128, ql * Wp:(ql + 1) * Wp]
                    )

                def load_convert_s2s_half(stg, lo, nsl):
                    # half-granular fill for block 0: 4 strip DMAs + converts
                    # + X2 build covering slots lo..lo+nsl only
                    for g in range(4):
                        nc.sync.dma_start(
                            stg[32 * g:32 * g + 32,
                                lo * W:(lo + nsl) * W].rearrange(
                                "p (s x) -> p s x", x=W
                            ),
                            fm_half[0, g, :, lo:lo + nsl, :],
                        )
                    if "cvt" in skip:
                        return
                    for j in range(lo, lo + nsl, 2):
                        nc.vector.tensor_copy(
                            xv[:, j:j + 2, 2:2 + W],
                            stg[:, j * W:(j + 2) * W].rearrange(
                                "p (s x) -> p s x", x=W
                            ),
                        )
                    s2s_range(lo, nsl)

                for _rep in range(repeat):
                  stg0 = spool.tile([128, SBLK * W], F32, tag="stg")
                  half = max(SBLK // 2, 1)
                  load_convert_s2s_half(stg0, 0, half)
                  if half < SBLK:
                      load_convert_s2s_half(stg0, half, SBLK - half)
                  stgs = {0: stg0}
                  otile = None
                  otiles = {}
                  SUB = 4
                  for u in range(T // SUB):
                      tsub = list(range(SUB * u, SUB * (u + 1)))
                      for t in tsub:
                          b = t // SBLK
                          if t % SBLK == 0:
                              if b + 1 < NBLK:
                                  stgs[b + 1] = load_block(b + 1)
                                  convert_block(stgs[b + 1], b + 1)
                                  s2s_block(b + 1)
                                  stgs.pop(b - 1, None)
                              else:
                                  s2s_epilogue()
                              otile = opool.tile([128, SBLK * W], F32, tag="ot")
                              otiles[b] = otile
                      if "mm" not in skip:
                          pts = {t: cps.tile([128, W], F32, name=f"pt{t}", tag="pt")
                                 for t in tsub}
                          info = []
                          for t in tsub:
                              for c in range(4):
                                  y = 4 * t + c
                                  info.append((t, c, y, pieces_for_row(y)))
                          nmm = {(t, c): 3 * len(p) for (t, c, _, p) in info}
                          seen = {(t, c): 0 for (t, c, _, p) in info}
                          # kx outer over the sub-block: each subarray keeps
                          # its weights for 2*SUB consecutive matmuls, and
                          # consecutive matmuls hit different subarrays.
                          # zigzag kx across sub-blocks so adjacent sub-blocks
                          # adjoin same-weight passes (one fewer weight swap
                          # per column per boundary).
                          kx_order = (0, 1, 2) if u % 2 == 0 else (2, 1, 0)
                          for kx in kx_order:
                              for (t, c, y, ps) in info:
                                  for (tape, g0, K, sl) in ps:
                                      a = ((y - 1) % 4) if tape == 0 \
                                          else ((y + 1) % 4)
                                      XT = X if tape == 0 else X2
                                      lhsT = Wt[
                                          32 * g0:32 * g0 + K,
                                          96 * a + 32 * kx:
                                          96 * a + 32 * kx + 32,
                                      ]
                                      rhs = XT[
                                          32 * g0:32 * g0 + K,
                                          sl * Wp + 1 + kx:
                                          sl * Wp + 1 + kx + W,
                                      ]
                                      seen[(t, c)] += 1
                                      nc.tensor.matmul(
                                          pts[t][32 * c:32 * c + 32, :],
                                          lhsT,
                                          rhs,
                                          start=seen[(t, c)] == 1,
                                          stop=seen[(t, c)] == nmm[(t, c)],
                                          tile_position=(32 * g0, 32 * c),
                                          skip_group_check=True,
                                      )
                      for t in tsub:
                          if "evac" not in skip:
                              # ACT only: the DVE must stay convert-only, or
                              # its in-order queue stalls next-block converts
                              # behind evacs that wait on matmuls
                              h = (t % SBLK) * W
                              nc.scalar.copy(
                                  otiles[t // SBLK][:, h:h + W], pts[t][:, :])
                          if "odma" not in skip and t % SBLK == SBLK - 1:
                              # outputs ride the SWDGE (gpsimd) queue so the
                              # ACT/DVE engines stay free for evacuations
                              for c in range(4):
                                  nc.gpsimd.dma_start(
                                      out_bs[t // SBLK, c],
                                      otiles[t // SBLK][
                                          32 * c:32 * c + 32, :
                                      ].rearrange("p (s x) -> p s x", x=W),
                                  )
                if timing:
                    s4 = opool.tile([1, NK], F32, tag="s4")
                    nc.sync.dma_start(s4[:, :], osink.ap()[0:1, 0, 0:NK])
                    nc.sync.dma_start(out.ap(), s4[:, :])

    nc.compile()
    return nc


_CACHE = {}


def _get_graph(H, W):
    key = (H, W)
    if key not in _CACHE:
        _CACHE[key] = build_graph(H, W)
    return _CACHE[key]


def kernel(fmap, mod, kernel_mod, weights):
    B, Ci, H, Wd = fmap.shape
    nc = _get_graph(H, Wd)
    in_maps = [
        {
            "fmap": np.ascontiguousarray(fmap[b], dtype=np.float32),
            "mod": np.ascontiguousarray(mod[b:b + 1], dtype=np.float32),
            "kernel_mod": np.ascontiguousarray(kernel_mod[b:b + 1], dtype=np.float32),
            "weights": np.ascontiguousarray(weights, dtype=np.float32),
        }
        for b in range(B)
    ]
    res = run_bass_kernel_spmd(nc, in_maps, core_ids=list(range(B)))
    return np.stack([res.results[b]["out"] for b in range(B)], axis=0)

